# revision 12
# baseline (speedup 1.0000x reference)
"""Trainium2 Bass kernel for MinimalEventMamba.

kernel(**inputs) takes FULL inputs (as from setup_inputs()) and returns the
FULL (4, 10, 64, 64) float32 output. Internally: batch-parallel across 8
NeuronCores (4 batches x2 replicated), one SPMD Bass kernel launch, host
assembles the output.

Per-core layout: channel-on-partition, L=4096 on free dim.
- encoder conv as im2col matmul (all 4 batches locally -> exact BN stats)
- mamba trunk: depthwise conv folded into in_proj taps (shifted matmuls,
  PSUM accumulate), dt_w folded into x_proj, softplus/silu/exp fused into
  PSUM evictions on ScalarE (per-partition scale/bias), selective scan via
  tensor_tensor_scan on DVE per state dim, B/C rows broadcast across
  partitions via DRAM-bounce DMA, y accumulation split DVE/GpSimd.
- decoder convs in zero-padded spatial layout (row stride 66) as 9-tap
  PSUM-accumulated matmuls; cross-batch BN stats via one tiny AllReduce.
"""
import sys
import types

sys.path.insert(0, "/opt/trn_rl_repo")
sys.path.insert(0, "/opt/trn_rl_repo/concourse")
try:
    from antenv import axon_hooks  # noqa: F401
except ImportError:
    try:
        from trn_agent_boot.trn_boot import _ntff_profile_via_ctypes
        _m = types.ModuleType("antenv.axon_hooks")
        _h = _ntff_profile_via_ctypes("/opt/axon/libaxon_pjrt.so")
        _m.get_axon_ntff_profile_hook = lambda: _h
        _m.set_axon_ntff_profile_hook = lambda h: None
        sys.modules["antenv.axon_hooks"] = _m
    except Exception:
        pass

from contextlib import ExitStack

import numpy as np
import ml_dtypes

import concourse.bass as bass
import concourse.tile as tile
from concourse import mybir
from concourse.bass_utils import run_bass_kernel_spmd
import bass_rust

F32 = mybir.dt.float32
F32R = mybir.dt.float32r
BF16 = mybir.dt.bfloat16
FP16 = mybir.dt.float16

NB, HD, NL, NF = 5, 64, 4, 10
DI, DS, DC, DTR = 128, 16, 4, 4
B, H, W = 4, 64, 64
L = H * W                     # 4096
PW = W + 2                    # padded row stride 66
PADL = PW * (H + 2) + 4       # padded spatial + guard cols (4360)
PBASE = 1 + PW + 1            # first interior col in padded layout
NCHUNK = 8                    # L / 512
CS = 512
LH = L // 2                   # s-loop half length


def split_excess_waits(nc, max_waits=1):
    """This container's walrus accepts only 1 sync wait per instruction;
    move overflow waits onto NOPs inserted before the offending op."""
    f = nc.m.functions[0]
    for bb in f.blocks:
        insts = bb.instructions
        i = 0
        while i < len(insts):
            inst = insts[i]
            si = inst.sync_info
            if si is not None and len(si.on_wait) > max_waits:
                waits = list(si.on_wait)
                si.on_wait = waits[-max_waits:]
                inst.sync_info = si
                overflow = waits[:-max_waits]
                eng = nc.engines[inst.engine]
                pos = i
                for j in range(0, len(overflow), max_waits):
                    nop = eng.nop(hint="splitw", nofuse=True)
                    nop_inst = nop.ins if hasattr(nop, "ins") else nop
                    for bb2 in f.blocks:
                        if any(x is nop_inst for x in bb2.instructions):
                            bb2.instructions[:] = [
                                x for x in bb2.instructions if x is not nop_inst
                            ]
                            break
                    nop_inst.sync_info = bass_rust.SyncInfo(
                        on_wait=overflow[j : j + max_waits], on_update=[]
                    )
                    insts.insert(pos, nop_inst)
                    pos += 1
                i = pos + 1
            else:
                i += 1


def _r(ap):
    return ap.bitcast(F32R)


def build_kernel():
    nc = bass.Bass()
    dp = nc.declare_dram_parameter

    enc_in = dp("enc_im2col", [45, B * L], FP16, isOutput=False)
    enc_w2 = dp("enc_w2", [45, HD], FP16, isOutput=False)
    enc_g = dp("enc_g", [HD, 1], F32, isOutput=False)
    enc_be = dp("enc_be", [HD, 1], F32, isOutput=False)
    ip_tap = dp("ip_tap", [HD, NL * DC * DI], FP16, isOutput=False)
    ip_z = dp("ip_z", [HD, NL * DI], FP16, isOutput=False)
    conv_b = dp("conv_b", [DI, NL], F32, isOutput=False)
    wd_T = dp("wd_T", [DI, NL * DI], BF16, isOutput=False)
    bc_T = dp("bc_T", [DI, NL * 2 * DS], BF16, isOutput=False)
    dt_b = dp("dt_b", [DI, NL], F32, isOutput=False)
    a_cols = dp("a_cols", [DI, NL * DS], F32, isOutput=False)
    d_col = dp("d_col", [DI, NL], F32, isOutput=False)
    op_T = dp("op_T", [DI, NL * HD], BF16, isOutput=False)
    dec1_tap = dp("dec1_tap", [HD, 9 * HD], FP16, isOutput=False)
    dec1_g = dp("dec1_g", [HD, 1], F32, isOutput=False)
    dec1_be = dp("dec1_be", [HD, 1], F32, isOutput=False)
    dec2_tap = dp("dec2_tap", [HD, 9 * NF], FP16, isOutput=False)
    dec2_b = dp("dec2_b", [NF, 1], F32, isOutput=False)

    out_ext = dp("out", [NF, L], F32, isOutput=True)

    bc_dram = nc.dram_tensor("bc_dram", [NL, 2 * DS, L], BF16)
    cc_in = nc.dram_tensor("cc_in", [HD, 2], F32)
    cc_out = nc.dram_tensor("cc_out", [HD, 2], F32, addr_space="Shared")

    ctx = ExitStack()
    with ctx:
        tc = ctx.enter_context(tile.TileContext(nc))
        const = ctx.enter_context(tc.tile_pool(name="const", bufs=1))
        persist = ctx.enter_context(tc.tile_pool(name="persist", bufs=1))
        work = ctx.enter_context(tc.tile_pool(name="work", bufs=1))
        stream = ctx.enter_context(tc.tile_pool(name="stream", bufs=2))
        sloop = ctx.enter_context(tc.tile_pool(name="sloop", bufs=2))
        small = ctx.enter_context(tc.tile_pool(name="small", bufs=1))
        psum = ctx.enter_context(tc.tile_pool(name="psum", bufs=6, space="PSUM"))
        psum1 = psum

        MM = nc.tensor.matmul
        AF = mybir.ActivationFunctionType
        OP = mybir.AluOpType
        X = mybir.AxisListType

        # ---------------- encoder ----------------
        enc_w_t = const.tile([45, HD], FP16)
        nc.sync.dma_start(enc_w_t[:], enc_w2[:])
        enc_g_t = const.tile([HD, 1], F32)
        nc.sync.dma_start(enc_g_t[:], enc_g[:])
        enc_be_t = const.tile([HD, 1], F32)
        nc.sync.dma_start(enc_be_t[:], enc_be[:])

        enc_keep = persist.tile([HD, L], F32)      # own-batch conv out
        s1p = small.tile([HD, 32], F32, tag="s1p")
        s2p = small.tile([HD, 32], F32, tag="s2p")
        for n in range(32):
            cin = stream.tile([45, CS], FP16, tag="enc_cin")
            nc.sync.dma_start(cin[:], enc_in[:, bass.ts(n, CS)])
            pt = psum.tile([HD, CS], F32, tag="mm512")
            MM(pt[:], enc_w_t[:], cin[:], start=True, stop=True)
            if n < NCHUNK:
                dst = enc_keep[:, bass.ts(n, CS)]
            else:
                scratch = stream.tile([HD, CS], F32, tag="enc_scr")
                dst = scratch[:]
            nc.scalar.activation(dst, pt[:], AF.Copy,
                                 accum_out=s1p[:, n : n + 1])
            sq = stream.tile([HD, CS], F32, tag="enc_sq")
            nc.scalar.activation(sq[:], pt[:], AF.Square,
                                 accum_out=s2p[:, n : n + 1])
        s1 = small.tile([HD, 1], F32, tag="s1")
        s2 = small.tile([HD, 1], F32, tag="s2")
        nc.vector.tensor_reduce(s1[:], s1p[:], axis=X.X, op=OP.add)
        nc.vector.tensor_reduce(s2[:], s2p[:], axis=X.X, op=OP.add)

        def bn_scale_bias(s1ap, s2ap, n_elems, g_ap, be_ap, tag):
            inv_n = 1.0 / n_elems
            mean = small.tile([HD, 1], F32, tag=tag + "m")
            nc.vector.tensor_scalar_mul(mean[:], s1ap, inv_n)
            m2 = small.tile([HD, 1], F32, tag=tag + "m2")
            nc.vector.tensor_tensor(m2[:], mean[:], mean[:], OP.mult)
            var = small.tile([HD, 1], F32, tag=tag + "v")
            nc.vector.scalar_tensor_tensor(var[:], s2ap, inv_n, m2[:],
                                           OP.mult, OP.subtract)
            veps = small.tile([HD, 1], F32, tag=tag + "ve")
            nc.vector.tensor_scalar_add(veps[:], var[:], 1e-5)
            rv = small.tile([HD, 1], F32, tag=tag + "rv")
            nc.vector.reciprocal(rv[:], veps[:])
            rstd = small.tile([HD, 1], F32, tag=tag + "rs")
            nc.scalar.activation(rstd[:], rv[:], AF.Sqrt)
            scale = small.tile([HD, 1], F32, tag=tag + "sc")
            nc.vector.tensor_tensor(scale[:], g_ap, rstd[:], OP.mult)
            nscale = small.tile([HD, 1], F32, tag=tag + "ns")
            nc.vector.tensor_scalar_mul(nscale[:], scale[:], -1.0)
            bias = small.tile([HD, 1], F32, tag=tag + "bi")
            nc.vector.scalar_tensor_tensor(bias[:], mean[:], nscale[:], be_ap,
                                           OP.mult, OP.add)
            return scale, bias

        sc0, bi0 = bn_scale_bias(s1[:], s2[:], B * L, enc_g_t[:], enc_be_t[:],
                                 "bn0")

        t_t = persist.tile([HD, 4 + L], F32)
        nc.vector.memset(t_t[:, 0:4], 0.0)
        nc.scalar.activation(t_t[:, 4:], enc_keep[:], AF.Relu,
                             bias=bi0[:], scale=sc0[:])
        t16 = persist.tile([HD, 4 + L], FP16)
        nc.vector.memset(t16[:, 0:4], 0.0)
        nc.scalar.activation(t16[:, 4:], t_t[:, 4:], AF.Copy)

        # ---------------- trunk weights ----------------
        iptap_t = const.tile([HD, NL * DC * DI], FP16)
        nc.sync.dma_start(iptap_t[:], ip_tap[:])
        ipz_t = const.tile([HD, NL * DI], FP16)
        nc.sync.dma_start(ipz_t[:], ip_z[:])
        convb_t = const.tile([DI, NL], F32)
        nc.sync.dma_start(convb_t[:], conv_b[:])
        wd_t = const.tile([DI, NL * DI], BF16)
        nc.sync.dma_start(wd_t[:], wd_T[:])
        bct_t = const.tile([DI, NL * 2 * DS], BF16)
        nc.sync.dma_start(bct_t[:], bc_T[:])
        dtb_t = const.tile([DI, NL], F32)
        nc.sync.dma_start(dtb_t[:], dt_b[:])
        acols_t = const.tile([DI, NL * DS], F32)
        nc.sync.dma_start(acols_t[:], a_cols[:])
        dcol_t = const.tile([DI, NL], F32)
        nc.sync.dma_start(dcol_t[:], d_col[:])
        opt_t = const.tile([DI, NL * HD], BF16)
        nc.sync.dma_start(opt_t[:], op_T[:])
        bc_sign = const.tile([2 * DS, 1], F32)
        nc.vector.memset(bc_sign[:], 1.0)
        nc.vector.memset(bc_sign[0:DS], -1.0)

        # ---------------- trunk ----------------
        for li in range(NL):
            xi_c = work.tile([DI, L], BF16, tag="xi_c")
            sz = work.tile([DI, L], BF16, tag="sz")
            dlt = work.tile([DI, L], BF16, tag="dlt")
            bc_sb = work.tile([2 * DS, L], BF16, tag="bc_sb")
            for n in range(NCHUNK):
                p_xi = psum.tile([DI, CS], F32, tag="mm512")
                for k in range(DC):
                    MM(p_xi[:],
                       iptap_t[:, (li * DC + k) * DI : (li * DC + k + 1) * DI],
                       t16[:, 1 + k + n * CS : 1 + k + n * CS + CS],
                       start=(k == 0), stop=(k == DC - 1))
                nc.scalar.activation(xi_c[:, bass.ts(n, CS)], p_xi[:], AF.Silu,
                                     bias=convb_t[:, li : li + 1], scale=1.0)
                p_z = psum.tile([DI, CS], F32, tag="mm512")
                MM(p_z[:], ipz_t[:, li * DI : (li + 1) * DI],
                   t16[:, 4 + n * CS : 4 + (n + 1) * CS],
                   start=True, stop=True)
                nc.scalar.activation(sz[:, bass.ts(n, CS)], p_z[:], AF.Silu)
            for n in range(NCHUNK):
                p_d = psum.tile([DI, CS], F32, tag="mm512")
                MM(p_d[:], wd_t[:, li * DI : (li + 1) * DI],
                   xi_c[:, bass.ts(n, CS)], start=True, stop=True)
                # delta = softplus(p_d + dt_b); store dlt = -delta = ln(sigmoid(-(p_d+dt_b)))
                sgm = stream.tile([DI, CS], F32, tag="sgm")
                nc.scalar.activation(sgm[:], p_d[:], AF.Sigmoid,
                                     bias=dtb_t[:, li : li + 1], scale=-1.0)
                nc.scalar.activation(dlt[:, bass.ts(n, CS)], sgm[:], AF.Ln)
                p_bc = psum1.tile([2 * DS, CS], F32, tag="mm512")
                MM(p_bc[:], bct_t[:, li * 2 * DS : (li + 1) * 2 * DS],
                   xi_c[:, bass.ts(n, CS)], start=True, stop=True)
                nc.scalar.activation(bc_sb[:, bass.ts(n, CS)], p_bc[:], AF.Copy,
                                     scale=bc_sign[:])
            nc.sync.dma_start(bc_dram[li], bc_sb[:])

            du = work.tile([DI, L], BF16, tag="du")
            nc.vector.tensor_tensor(du[:], dlt[:], xi_c[:], OP.mult)
            y = work.tile([DI, L], BF16, tag="y")
            nc.vector.tensor_scalar_mul(y[:], xi_c[:], dcol_t[:, li : li + 1])

            for s in range(DS):
                hs_prev = None
                for hf in range(2):
                    sl = slice(hf * LH, (hf + 1) * LH)
                    dA = sloop.tile([DI, LH], FP16, tag="dA")
                    nc.scalar.activation(
                        dA[:], dlt[:, sl], AF.Exp,
                        scale=acols_t[:, li * DS + s : li * DS + s + 1])
                    brep = sloop.tile([DI, LH], BF16, tag="brep")
                    nc.sync.dma_start(
                        brep[:],
                        bc_dram[li][s : s + 1, sl].broadcast_to((DI, LH)))
                    crep = sloop.tile([DI, LH], BF16, tag="crep")
                    nc.sync.dma_start(
                        crep[:],
                        bc_dram[li][DS + s : DS + s + 1, sl].broadcast_to(
                            (DI, LH)))
                    xs = sloop.tile([DI, LH], BF16, tag="xs")
                    nc.vector.tensor_tensor(xs[:], du[:, sl], brep[:], OP.mult)
                    hs = sloop.tile([DI, LH], BF16, tag="hs")
                    init = 0.0 if hf == 0 else hs_prev[:, LH - 1 : LH]
                    nc.vector.tensor_tensor_scan(hs[:], dA[:], xs[:], init,
                                                 OP.mult, OP.add)
                    hs_prev = hs
                    hc = sloop.tile([DI, LH], BF16, tag="hc")
                    nc.gpsimd.tensor_tensor(hc[:], hs[:], crep[:], OP.mult)
                    nc.vector.tensor_tensor(y[:, sl], y[:, sl], hc[:], OP.add)

            yg = work.tile([DI, L], BF16, tag="yg")
            nc.vector.tensor_tensor(yg[:], y[:], sz[:], OP.mult)
            for n in range(NCHUNK):
                p_o = psum1.tile([HD, CS], F32, tag="mm512")
                MM(p_o[:], opt_t[:, li * HD : (li + 1) * HD],
                   yg[:, bass.ts(n, CS)], start=True, stop=True)
                nc.vector.tensor_tensor(
                    t_t[:, 4 + n * CS : 4 + (n + 1) * CS],
                    t_t[:, 4 + n * CS : 4 + (n + 1) * CS], p_o[:], OP.add)
                if li < NL - 1:
                    nc.scalar.activation(
                        t16[:, 4 + n * CS : 4 + (n + 1) * CS],
                        t_t[:, 4 + n * CS : 4 + (n + 1) * CS], AF.Copy)

        # ---------------- decoder ----------------
        d1_taps = const.tile([HD, 9 * HD], FP16)
        nc.sync.dma_start(d1_taps[:], dec1_tap[:])
        d2_taps = const.tile([HD, 9 * NF], FP16)
        nc.sync.dma_start(d2_taps[:], dec2_tap[:])
        d1g_t = const.tile([HD, 1], F32)
        nc.sync.dma_start(d1g_t[:], dec1_g[:])
        d1be_t = const.tile([HD, 1], F32)
        nc.sync.dma_start(d1be_t[:], dec1_be[:])
        d2b_t = const.tile([NF, 1], F32)
        nc.sync.dma_start(d2b_t[:], dec2_b[:])

        padA = persist.tile([HD, PADL], FP16)
        nc.vector.memset(padA[:], 0.0)
        padB = persist.tile([HD, PADL], FP16)
        nc.vector.memset(padB[:], 0.0)
        out_pad = persist.tile([NF, PADL], F32)

        def interior(tile_ap):
            return tile_ap[:, PBASE : PBASE + PW * H].rearrange(
                "p (h w) -> p h w", w=PW)[:, :, 0:W]

        nc.scalar.activation(interior(padA),
                             t_t[:, 4:].rearrange("p (h w) -> p h w", w=W),
                             AF.Copy)

        def conv9(dst_tile, src_tile, taps_tile, m_out, tapw, evict):
            total = PW * H
            nch = (total + CS - 1) // CS
            for n in range(nch):
                c0 = PBASE + n * CS
                cw = min(CS, PBASE + total - c0)
                pt = psum1.tile([m_out, CS], F32, tag="mm512")
                for ti in range(9):
                    dy, dx = ti // 3, ti % 3
                    off = c0 + (dy - 1) * PW + (dx - 1)
                    MM(pt[:, 0:cw],
                       taps_tile[:, ti * tapw : ti * tapw + m_out],
                       src_tile[:, off : off + cw],
                       start=(ti == 0), stop=(ti == 8))
                evict(dst_tile[0:m_out, c0 : c0 + cw], pt[:, 0:cw])

        conv9(padB, padA, d1_taps, HD, HD,
              lambda d, p: nc.scalar.activation(d, p, AF.Copy))

        d1_int = interior(padB)
        ds1 = small.tile([HD, 1], F32, tag="ds1")
        nc.vector.tensor_reduce(ds1[:], d1_int, axis=X.XY, op=OP.add)
        ds2 = small.tile([HD, 1], F32, tag="ds2")
        nc.scalar.activation(interior(padA), d1_int, AF.Square,
                             accum_out=ds2[:])
        packed = small.tile([HD, 2], F32, tag="pk")
        nc.vector.tensor_copy(packed[:, 0:1], ds1[:])
        nc.vector.tensor_copy(packed[:, 1:2], ds2[:])
        nc.sync.dma_start(cc_in[:], packed[:])
        nc.gpsimd.collective_compute(
            "AllReduce", OP.add, replica_groups=[list(range(8))],
            ins=[cc_in[:]], outs=[cc_out[:]])
        red = small.tile([HD, 2], F32, tag="red")
        nc.sync.dma_start(red[:], cc_out[:])
        sc1, bi1 = bn_scale_bias(red[:, 0:1], red[:, 1:2], 2 * B * L,
                                 d1g_t[:], d1be_t[:], "bn1")

        # h2 into padA (pads still zero outside interior; square scratch
        # gets overwritten)
        nc.scalar.activation(interior(padA), d1_int, AF.Relu,
                             bias=bi1[:], scale=sc1[:])
        # conv2 into padB (its pads hold stale conv1 data only at pad
        # columns; interior fully rewritten; final DMA reads interior only)
        conv9(out_pad, padA, d2_taps, NF, NF,
              lambda d, p: nc.scalar.activation(
                  d, p, AF.Identity, bias=d2b_t[:], scale=1.0))
        out_int = out_pad[:NF, PBASE : PBASE + PW * H].rearrange(
            "p (h w) -> p h w", w=PW)[:, :, 0:W]
        nc.sync.dma_start(out_ext[:].rearrange("p (h w) -> p h w", w=W),
                          out_int)

    split_excess_waits(nc)
    return nc


_CACHED = {}


def _get_kernel():
    if "nc" not in _CACHED:
        _CACHED["nc"] = build_kernel()
    return _CACHED["nc"]


def _host_inputs(inputs):
    f32 = np.float32
    bf16 = ml_dtypes.bfloat16
    x = np.asarray(inputs["x"], f32)
    enc_w = np.asarray(inputs["enc_w"], f32)
    in_proj = np.asarray(inputs["in_proj"], f32)
    conv_w = np.asarray(inputs["conv_w"], f32)
    x_proj = np.asarray(inputs["x_proj"], f32)
    dt_w = np.asarray(inputs["dt_w"], f32)
    A_log = np.asarray(inputs["A_log"], f32)
    out_proj = np.asarray(inputs["out_proj"], f32)
    dec1_w = np.asarray(inputs["dec1_w"], f32)
    dec2_w = np.asarray(inputs["dec2_w"], f32)

    xp = np.zeros((B, NB, H + 2, W + 2), f32)
    xp[:, :, 1:-1, 1:-1] = x
    cols = np.empty((NB, 3, 3, B, L), f32)
    for dy in range(3):
        for dx in range(3):
            cols[:, dy, dx] = (
                xp[:, :, dy : dy + H, dx : dx + W]
                .reshape(B, NB, L).transpose(1, 0, 2))
    cols_b = cols.reshape(45, B, L)
    enc_w2 = np.ascontiguousarray(enc_w.reshape(HD, 45).T)

    ip_tap = np.empty((HD, NL, DC, DI), f32)
    ip_z = np.empty((HD, NL, DI), f32)
    wd_T = np.empty((DI, NL, DI), f32)
    bc_T = np.empty((DI, NL, 2 * DS), f32)
    a_cols = np.empty((DI, NL, DS), f32)
    op_T = np.empty((DI, NL, HD), f32)
    for i in range(NL):
        for k in range(DC):
            ip_tap[:, i, k, :] = (conv_w[i][:, k : k + 1] * in_proj[i][:DI]).T
        ip_z[:, i, :] = in_proj[i][DI:].T
        wd_T[:, i, :] = (dt_w[i] @ x_proj[i][:DTR]).T
        bc_T[:, i, :] = x_proj[i][DTR:].T
        a_cols[:, i, :] = np.exp(A_log[i])
        op_T[:, i, :] = out_proj[i].T

    dec1_tap = np.empty((HD, 9, HD), f32)
    dec2_tap = np.empty((HD, 9, NF), f32)
    for ti in range(9):
        dy, dx = ti // 3, ti % 3
        dec1_tap[:, ti, :] = dec1_w[:, :, dy, dx].T
        dec2_tap[:, ti, :] = dec2_w[:, :, dy, dx].T

    common = {
        "enc_w2": enc_w2.astype(np.float16),
        "enc_g": np.asarray(inputs["enc_g"], f32).reshape(HD, 1),
        "enc_be": np.asarray(inputs["enc_be"], f32).reshape(HD, 1),
        "ip_tap": ip_tap.reshape(HD, NL * DC * DI).astype(np.float16),
        "ip_z": ip_z.reshape(HD, NL * DI).astype(np.float16),
        "conv_b": np.ascontiguousarray(
            np.asarray(inputs["conv_b"], f32).T),           # (DI, NL)
        "wd_T": wd_T.reshape(DI, NL * DI).astype(bf16),
        "bc_T": bc_T.reshape(DI, NL * 2 * DS).astype(bf16),
        "dt_b": np.ascontiguousarray(-np.asarray(inputs["dt_b"], f32).T),
        "a_cols": a_cols.reshape(DI, NL * DS),
        "d_col": np.ascontiguousarray(np.asarray(inputs["Dp"], f32).T),
        "op_T": op_T.reshape(DI, NL * HD).astype(bf16),
        "dec1_tap": dec1_tap.reshape(HD, 9 * HD).astype(np.float16),
        "dec1_g": np.asarray(inputs["dec1_g"], f32).reshape(HD, 1),
        "dec1_be": np.asarray(inputs["dec1_be"], f32).reshape(HD, 1),
        "dec2_tap": dec2_tap.reshape(HD, 9 * NF).astype(np.float16),
        "dec2_b": np.asarray(inputs["dec2_b"], f32).reshape(NF, 1),
    }
    in_maps = []
    for c in range(8):
        b0 = c % B
        order = [b0] + [bb for bb in range(B) if bb != b0]
        m = dict(common)
        m["enc_im2col"] = np.ascontiguousarray(
            cols_b[:, order, :].reshape(45, B * L)).astype(np.float16)
        in_maps.append(m)
    return in_maps


def kernel(**inputs):
    nc = _get_kernel()
    in_maps = _host_inputs(inputs)
    res = run_bass_kernel_spmd(nc, in_maps, core_ids=list(range(8)))
    out = np.empty((B, NF, H, W), np.float32)
    for b_ in range(B):
        out[b_] = res.results[b_]["out"].reshape(NF, H, W)
    return out


if __name__ == "__main__":
    sys.path.insert(0, "/root/problem")
    import reference as ref

    inp = {k: np.asarray(v) for k, v in ref.setup_inputs().items()}
    got = kernel(**inp)
    print("kernel ran, output shape:", got.shape)


# revision 14
# speedup vs baseline: 1.0019x; 1.0019x over previous
"""Trainium2 Bass kernel for MinimalEventMamba.

kernel(**inputs) takes FULL inputs (as from setup_inputs()) and returns the
FULL (4, 10, 64, 64) float32 output. Internally: batch-parallel across 8
NeuronCores (4 batches x2 replicated), one SPMD Bass kernel launch, host
assembles the output.

Per-core layout: channel-on-partition, L=4096 on free dim.
- encoder conv as im2col matmul (all 4 batches locally -> exact BN stats)
- mamba trunk: depthwise conv folded into in_proj taps (shifted matmuls,
  PSUM accumulate), dt_w folded into x_proj, softplus/silu/exp fused into
  PSUM evictions on ScalarE (per-partition scale/bias), selective scan via
  tensor_tensor_scan on DVE per state dim, B/C rows broadcast across
  partitions via DRAM-bounce DMA, y accumulation split DVE/GpSimd.
- decoder convs in zero-padded spatial layout (row stride 66) as 9-tap
  PSUM-accumulated matmuls; cross-batch BN stats via one tiny AllReduce.
"""
import sys
import types

sys.path.insert(0, "/opt/trn_rl_repo")
sys.path.insert(0, "/opt/trn_rl_repo/concourse")
try:
    from antenv import axon_hooks  # noqa: F401
except ImportError:
    try:
        from trn_agent_boot.trn_boot import _ntff_profile_via_ctypes
        _m = types.ModuleType("antenv.axon_hooks")
        _h = _ntff_profile_via_ctypes("/opt/axon/libaxon_pjrt.so")
        _m.get_axon_ntff_profile_hook = lambda: _h
        _m.set_axon_ntff_profile_hook = lambda h: None
        sys.modules["antenv.axon_hooks"] = _m
    except Exception:
        pass

from contextlib import ExitStack

import numpy as np
import ml_dtypes

import concourse.bass as bass
import concourse.tile as tile
from concourse import mybir
from concourse.bass_utils import run_bass_kernel_spmd
import bass_rust

F32 = mybir.dt.float32
F32R = mybir.dt.float32r
BF16 = mybir.dt.bfloat16
FP16 = mybir.dt.float16

NB, HD, NL, NF = 5, 64, 4, 10
DI, DS, DC, DTR = 128, 16, 4, 4
B, H, W = 4, 64, 64
L = H * W                     # 4096
PW = W + 2                    # padded row stride 66
PADL = PW * (H + 2) + 4       # padded spatial + guard cols (4360)
PBASE = 1 + PW + 1            # first interior col in padded layout
NCHUNK = 8                    # L / 512
CS = 512
LH = L // 2                   # s-loop half length


def split_excess_waits(nc, max_waits=1):
    """This container's walrus accepts only 1 sync wait per instruction;
    move overflow waits onto NOPs inserted before the offending op."""
    f = nc.m.functions[0]
    for bb in f.blocks:
        insts = bb.instructions
        i = 0
        while i < len(insts):
            inst = insts[i]
            si = inst.sync_info
            if si is not None and len(si.on_wait) > max_waits:
                waits = list(si.on_wait)
                si.on_wait = waits[-max_waits:]
                inst.sync_info = si
                overflow = waits[:-max_waits]
                eng = nc.engines[inst.engine]
                pos = i
                for j in range(0, len(overflow), max_waits):
                    nop = eng.nop(hint="splitw", nofuse=True)
                    nop_inst = nop.ins if hasattr(nop, "ins") else nop
                    for bb2 in f.blocks:
                        if any(x is nop_inst for x in bb2.instructions):
                            bb2.instructions[:] = [
                                x for x in bb2.instructions if x is not nop_inst
                            ]
                            break
                    nop_inst.sync_info = bass_rust.SyncInfo(
                        on_wait=overflow[j : j + max_waits], on_update=[]
                    )
                    insts.insert(pos, nop_inst)
                    pos += 1
                i = pos + 1
            else:
                i += 1


def _r(ap):
    return ap.bitcast(F32R)


def build_kernel():
    nc = bass.Bass()
    dp = nc.declare_dram_parameter

    enc_in = dp("enc_im2col", [45, B * L], FP16, isOutput=False)
    enc_w2 = dp("enc_w2", [45, HD], FP16, isOutput=False)
    enc_g = dp("enc_g", [HD, 1], F32, isOutput=False)
    enc_be = dp("enc_be", [HD, 1], F32, isOutput=False)
    ip_tap = dp("ip_tap", [HD, NL * DC * DI], FP16, isOutput=False)
    ip_z = dp("ip_z", [HD, NL * DI], FP16, isOutput=False)
    conv_b = dp("conv_b", [DI, NL], F32, isOutput=False)
    wd_T = dp("wd_T", [DI, NL * DI], BF16, isOutput=False)
    bc_T = dp("bc_T", [DI, NL * 2 * DS], BF16, isOutput=False)
    dt_b = dp("dt_b", [DI, NL], F32, isOutput=False)
    a_cols = dp("a_cols", [DI, NL * DS], F32, isOutput=False)
    d_col = dp("d_col", [DI, NL], F32, isOutput=False)
    op_T = dp("op_T", [DI, NL * HD], BF16, isOutput=False)
    dec1_tap = dp("dec1_tap", [HD, 9 * HD], FP16, isOutput=False)
    dec1_g = dp("dec1_g", [HD, 1], F32, isOutput=False)
    dec1_be = dp("dec1_be", [HD, 1], F32, isOutput=False)
    dec2_tap = dp("dec2_tap", [HD, 9 * NF], FP16, isOutput=False)
    dec2_b = dp("dec2_b", [NF, 1], F32, isOutput=False)

    out_ext = dp("out", [NF, L], F32, isOutput=True)

    bc_dram = nc.dram_tensor("bc_dram", [NL, 2 * DS, L], BF16)
    cc_in = nc.dram_tensor("cc_in", [HD, 2], F32)
    cc_out = nc.dram_tensor("cc_out", [HD, 2], F32, addr_space="Shared")

    ctx = ExitStack()
    with ctx:
        tc = ctx.enter_context(tile.TileContext(nc))
        const = ctx.enter_context(tc.tile_pool(name="const", bufs=1))
        persist = ctx.enter_context(tc.tile_pool(name="persist", bufs=1))
        work = ctx.enter_context(tc.tile_pool(name="work", bufs=1))
        stream = ctx.enter_context(tc.tile_pool(name="stream", bufs=2))
        sloop = ctx.enter_context(tc.tile_pool(name="sloop", bufs=2))
        small = ctx.enter_context(tc.tile_pool(name="small", bufs=1))
        psum = ctx.enter_context(tc.tile_pool(name="psum", bufs=6, space="PSUM"))
        psum1 = psum

        MM = nc.tensor.matmul
        AF = mybir.ActivationFunctionType
        OP = mybir.AluOpType
        X = mybir.AxisListType

        # ---------------- encoder ----------------
        enc_w_t = const.tile([45, HD], FP16)
        nc.sync.dma_start(enc_w_t[:], enc_w2[:])
        enc_g_t = const.tile([HD, 1], F32)
        nc.sync.dma_start(enc_g_t[:], enc_g[:])
        enc_be_t = const.tile([HD, 1], F32)
        nc.sync.dma_start(enc_be_t[:], enc_be[:])

        enc_keep = persist.tile([HD, L], F32)      # own-batch conv out
        s1p = small.tile([HD, 32], F32, tag="s1p")
        s2p = small.tile([HD, 32], F32, tag="s2p")
        for n in range(32):
            cin = stream.tile([45, CS], FP16, tag="enc_cin")
            nc.sync.dma_start(cin[:], enc_in[:, bass.ts(n, CS)])
            pt = psum.tile([HD, CS], F32, tag="mm512")
            MM(pt[:], enc_w_t[:], cin[:], start=True, stop=True)
            if n < NCHUNK:
                dst = enc_keep[:, bass.ts(n, CS)]
            else:
                scratch = stream.tile([HD, CS], F32, tag="enc_scr")
                dst = scratch[:]
            nc.scalar.activation(dst, pt[:], AF.Copy,
                                 accum_out=s1p[:, n : n + 1])
            sq = stream.tile([HD, CS], F32, tag="enc_scr")
            nc.scalar.activation(sq[:], pt[:], AF.Square,
                                 accum_out=s2p[:, n : n + 1])
        s1 = small.tile([HD, 1], F32, tag="s1")
        s2 = small.tile([HD, 1], F32, tag="s2")
        nc.vector.tensor_reduce(s1[:], s1p[:], axis=X.X, op=OP.add)
        nc.vector.tensor_reduce(s2[:], s2p[:], axis=X.X, op=OP.add)

        def bn_scale_bias(s1ap, s2ap, n_elems, g_ap, be_ap, tag):
            inv_n = 1.0 / n_elems
            mean = small.tile([HD, 1], F32, tag=tag + "m")
            nc.vector.tensor_scalar_mul(mean[:], s1ap, inv_n)
            m2 = small.tile([HD, 1], F32, tag=tag + "m2")
            nc.vector.tensor_tensor(m2[:], mean[:], mean[:], OP.mult)
            var = small.tile([HD, 1], F32, tag=tag + "v")
            nc.vector.scalar_tensor_tensor(var[:], s2ap, inv_n, m2[:],
                                           OP.mult, OP.subtract)
            veps = small.tile([HD, 1], F32, tag=tag + "ve")
            nc.vector.tensor_scalar_add(veps[:], var[:], 1e-5)
            rv = small.tile([HD, 1], F32, tag=tag + "rv")
            nc.vector.reciprocal(rv[:], veps[:])
            rstd = small.tile([HD, 1], F32, tag=tag + "rs")
            nc.scalar.activation(rstd[:], rv[:], AF.Sqrt)
            scale = small.tile([HD, 1], F32, tag=tag + "sc")
            nc.vector.tensor_tensor(scale[:], g_ap, rstd[:], OP.mult)
            nscale = small.tile([HD, 1], F32, tag=tag + "ns")
            nc.vector.tensor_scalar_mul(nscale[:], scale[:], -1.0)
            bias = small.tile([HD, 1], F32, tag=tag + "bi")
            nc.vector.scalar_tensor_tensor(bias[:], mean[:], nscale[:], be_ap,
                                           OP.mult, OP.add)
            return scale, bias

        sc0, bi0 = bn_scale_bias(s1[:], s2[:], B * L, enc_g_t[:], enc_be_t[:],
                                 "bn0")

        t_t = persist.tile([HD, 4 + L], F32)
        nc.vector.memset(t_t[:, 0:4], 0.0)
        nc.scalar.activation(t_t[:, 4:], enc_keep[:], AF.Relu,
                             bias=bi0[:], scale=sc0[:])
        t16 = persist.tile([HD, 4 + L], FP16)
        nc.vector.memset(t16[:, 0:4], 0.0)
        nc.scalar.activation(t16[:, 4:], t_t[:, 4:], AF.Copy)

        # ---------------- trunk weights ----------------
        iptap_t = const.tile([HD, NL * DC * DI], FP16)
        nc.sync.dma_start(iptap_t[:], ip_tap[:])
        ipz_t = const.tile([HD, NL * DI], FP16)
        nc.sync.dma_start(ipz_t[:], ip_z[:])
        convb_t = const.tile([DI, NL], F32)
        nc.sync.dma_start(convb_t[:], conv_b[:])
        wd_t = const.tile([DI, NL * DI], BF16)
        nc.sync.dma_start(wd_t[:], wd_T[:])
        bct_t = const.tile([DI, NL * 2 * DS], BF16)
        nc.sync.dma_start(bct_t[:], bc_T[:])
        dtb_t = const.tile([DI, NL], F32)
        nc.sync.dma_start(dtb_t[:], dt_b[:])
        acols_t = const.tile([DI, NL * DS], F32)
        nc.sync.dma_start(acols_t[:], a_cols[:])
        dcol_t = const.tile([DI, NL], F32)
        nc.sync.dma_start(dcol_t[:], d_col[:])
        opt_t = const.tile([DI, NL * HD], BF16)
        nc.sync.dma_start(opt_t[:], op_T[:])
        bc_sign = const.tile([2 * DS, 1], F32)
        nc.vector.memset(bc_sign[:], 1.0)
        nc.vector.memset(bc_sign[0:DS], -1.0)

        # ---------------- trunk ----------------
        for li in range(NL):
            xi_c = work.tile([DI, L], BF16, tag="xi_c")
            sz = work.tile([DI, L], BF16, tag="sz")
            dlt = work.tile([DI, L], BF16, tag="dlt")
            bc_sb = work.tile([2 * DS, L], BF16, tag="bc_sb")
            for n in range(NCHUNK):
                p_xi = psum.tile([DI, CS], F32, tag="mm512")
                for k in range(DC):
                    MM(p_xi[:],
                       iptap_t[:, (li * DC + k) * DI : (li * DC + k + 1) * DI],
                       t16[:, 1 + k + n * CS : 1 + k + n * CS + CS],
                       start=(k == 0), stop=(k == DC - 1))
                nc.scalar.activation(xi_c[:, bass.ts(n, CS)], p_xi[:], AF.Silu,
                                     bias=convb_t[:, li : li + 1], scale=1.0)
                p_z = psum.tile([DI, CS], F32, tag="mm512")
                MM(p_z[:], ipz_t[:, li * DI : (li + 1) * DI],
                   t16[:, 4 + n * CS : 4 + (n + 1) * CS],
                   start=True, stop=True)
                nc.scalar.activation(sz[:, bass.ts(n, CS)], p_z[:], AF.Silu)
            for n in range(NCHUNK):
                p_d = psum.tile([DI, CS], F32, tag="mm512")
                MM(p_d[:], wd_t[:, li * DI : (li + 1) * DI],
                   xi_c[:, bass.ts(n, CS)], start=True, stop=True)
                # delta = softplus(p_d + dt_b); store dlt = -delta = ln(sigmoid(-(p_d+dt_b)))
                sgm = stream.tile([DI, CS], F32, tag="sgm")
                nc.scalar.activation(sgm[:], p_d[:], AF.Sigmoid,
                                     bias=dtb_t[:, li : li + 1], scale=-1.0)
                nc.scalar.activation(dlt[:, bass.ts(n, CS)], sgm[:], AF.Ln)
                p_bc = psum1.tile([2 * DS, CS], F32, tag="mm512")
                MM(p_bc[:], bct_t[:, li * 2 * DS : (li + 1) * 2 * DS],
                   xi_c[:, bass.ts(n, CS)], start=True, stop=True)
                nc.scalar.activation(bc_sb[:, bass.ts(n, CS)], p_bc[:], AF.Copy,
                                     scale=bc_sign[:])
            nc.sync.dma_start(bc_dram[li], bc_sb[:])

            du = work.tile([DI, L], BF16, tag="du")
            nc.vector.tensor_tensor(du[:], dlt[:], xi_c[:], OP.mult)
            # per-half ping-pong accumulators (out-of-place keeps DVE 2x mode)
            acc = [None, None]
            for hf in range(2):
                a0 = sloop.tile([DI, LH], BF16, tag=f"acc{hf}")
                nc.vector.tensor_scalar_mul(
                    a0[:], xi_c[:, hf * LH : (hf + 1) * LH],
                    dcol_t[:, li : li + 1])
                acc[hf] = a0

            for s in range(DS):
                hs_prev = None
                for hf in range(2):
                    sl = slice(hf * LH, (hf + 1) * LH)
                    dA = sloop.tile([DI, LH], FP16, tag="dA")
                    nc.scalar.activation(
                        dA[:], dlt[:, sl], AF.Exp,
                        scale=acols_t[:, li * DS + s : li * DS + s + 1])
                    brep = sloop.tile([DI, LH], BF16, tag="brep")
                    nc.sync.dma_start(
                        brep[:],
                        bc_dram[li][s : s + 1, sl].broadcast_to((DI, LH)))
                    crep = sloop.tile([DI, LH], BF16, tag="crep")
                    nc.sync.dma_start(
                        crep[:],
                        bc_dram[li][DS + s : DS + s + 1, sl].broadcast_to(
                            (DI, LH)))
                    xs = sloop.tile([DI, LH], BF16, tag="xs")
                    nc.vector.tensor_tensor(xs[:], du[:, sl], brep[:], OP.mult)
                    hs = sloop.tile([DI, LH], BF16, tag="hs")
                    init = 0.0 if hf == 0 else hs_prev[:, LH - 1 : LH]
                    nc.vector.tensor_tensor_scan(hs[:], dA[:], xs[:], init,
                                                 OP.mult, OP.add)
                    hs_prev = hs
                    hc = sloop.tile([DI, LH], BF16, tag="hc")
                    nc.gpsimd.tensor_tensor(hc[:], hs[:], crep[:], OP.mult)
                    anew = sloop.tile([DI, LH], BF16, tag=f"acc{hf}")
                    nc.vector.tensor_tensor(anew[:], acc[hf][:], hc[:], OP.add)
                    acc[hf] = anew

            yg = work.tile([DI, L], BF16, tag="yg")
            for hf in range(2):
                sl = slice(hf * LH, (hf + 1) * LH)
                nc.vector.tensor_tensor(yg[:, sl], acc[hf][:], sz[:, sl],
                                        OP.mult)
            for n in range(NCHUNK):
                p_o = psum1.tile([HD, CS], F32, tag="mm512")
                MM(p_o[:], opt_t[:, li * HD : (li + 1) * HD],
                   yg[:, bass.ts(n, CS)], start=True, stop=True)
                nc.vector.tensor_tensor(
                    t_t[:, 4 + n * CS : 4 + (n + 1) * CS],
                    t_t[:, 4 + n * CS : 4 + (n + 1) * CS], p_o[:], OP.add)
                if li < NL - 1:
                    nc.scalar.activation(
                        t16[:, 4 + n * CS : 4 + (n + 1) * CS],
                        t_t[:, 4 + n * CS : 4 + (n + 1) * CS], AF.Copy)

        # ---------------- decoder ----------------
        d1_taps = const.tile([HD, 9 * HD], FP16)
        nc.sync.dma_start(d1_taps[:], dec1_tap[:])
        d2_taps = const.tile([HD, 9 * NF], FP16)
        nc.sync.dma_start(d2_taps[:], dec2_tap[:])
        d1g_t = const.tile([HD, 1], F32)
        nc.sync.dma_start(d1g_t[:], dec1_g[:])
        d1be_t = const.tile([HD, 1], F32)
        nc.sync.dma_start(d1be_t[:], dec1_be[:])
        d2b_t = const.tile([NF, 1], F32)
        nc.sync.dma_start(d2b_t[:], dec2_b[:])

        padA = persist.tile([HD, PADL], FP16)
        nc.vector.memset(padA[:], 0.0)
        padB = persist.tile([HD, PADL], FP16)
        nc.vector.memset(padB[:], 0.0)
        out_pad = persist.tile([NF, PADL], F32)

        def interior(tile_ap):
            return tile_ap[:, PBASE : PBASE + PW * H].rearrange(
                "p (h w) -> p h w", w=PW)[:, :, 0:W]

        nc.scalar.activation(interior(padA),
                             t_t[:, 4:].rearrange("p (h w) -> p h w", w=W),
                             AF.Copy)

        def conv9(dst_tile, src_tile, taps_tile, m_out, tapw, evict):
            total = PW * H
            nch = (total + CS - 1) // CS
            for n in range(nch):
                c0 = PBASE + n * CS
                cw = min(CS, PBASE + total - c0)
                pt = psum1.tile([m_out, CS], F32, tag="mm512")
                for ti in range(9):
                    dy, dx = ti // 3, ti % 3
                    off = c0 + (dy - 1) * PW + (dx - 1)
                    MM(pt[:, 0:cw],
                       taps_tile[:, ti * tapw : ti * tapw + m_out],
                       src_tile[:, off : off + cw],
                       start=(ti == 0), stop=(ti == 8))
                evict(dst_tile[0:m_out, c0 : c0 + cw], pt[:, 0:cw])

        conv9(padB, padA, d1_taps, HD, HD,
              lambda d, p: nc.scalar.activation(d, p, AF.Copy))

        d1_int = interior(padB)
        ds1 = small.tile([HD, 1], F32, tag="ds1")
        nc.vector.tensor_reduce(ds1[:], d1_int, axis=X.XY, op=OP.add)
        ds2 = small.tile([HD, 1], F32, tag="ds2")
        nc.scalar.activation(interior(padA), d1_int, AF.Square,
                             accum_out=ds2[:])
        packed = small.tile([HD, 2], F32, tag="pk")
        nc.vector.tensor_copy(packed[:, 0:1], ds1[:])
        nc.vector.tensor_copy(packed[:, 1:2], ds2[:])
        nc.sync.dma_start(cc_in[:], packed[:])
        nc.gpsimd.collective_compute(
            "AllReduce", OP.add, replica_groups=[list(range(8))],
            ins=[cc_in[:]], outs=[cc_out[:]])
        red = small.tile([HD, 2], F32, tag="red")
        nc.sync.dma_start(red[:], cc_out[:])
        sc1, bi1 = bn_scale_bias(red[:, 0:1], red[:, 1:2], 2 * B * L,
                                 d1g_t[:], d1be_t[:], "bn1")

        # h2 into padA (pads still zero outside interior; square scratch
        # gets overwritten)
        nc.scalar.activation(interior(padA), d1_int, AF.Relu,
                             bias=bi1[:], scale=sc1[:])
        # conv2 into padB (its pads hold stale conv1 data only at pad
        # columns; interior fully rewritten; final DMA reads interior only)
        conv9(out_pad, padA, d2_taps, NF, NF,
              lambda d, p: nc.scalar.activation(
                  d, p, AF.Identity, bias=d2b_t[:], scale=1.0))
        out_int = out_pad[:NF, PBASE : PBASE + PW * H].rearrange(
            "p (h w) -> p h w", w=PW)[:, :, 0:W]
        nc.sync.dma_start(out_ext[:].rearrange("p (h w) -> p h w", w=W),
                          out_int)

    split_excess_waits(nc)
    return nc


_CACHED = {}


def _get_kernel():
    if "nc" not in _CACHED:
        _CACHED["nc"] = build_kernel()
    return _CACHED["nc"]


def _host_inputs(inputs):
    f32 = np.float32
    bf16 = ml_dtypes.bfloat16
    x = np.asarray(inputs["x"], f32)
    enc_w = np.asarray(inputs["enc_w"], f32)
    in_proj = np.asarray(inputs["in_proj"], f32)
    conv_w = np.asarray(inputs["conv_w"], f32)
    x_proj = np.asarray(inputs["x_proj"], f32)
    dt_w = np.asarray(inputs["dt_w"], f32)
    A_log = np.asarray(inputs["A_log"], f32)
    out_proj = np.asarray(inputs["out_proj"], f32)
    dec1_w = np.asarray(inputs["dec1_w"], f32)
    dec2_w = np.asarray(inputs["dec2_w"], f32)

    xp = np.zeros((B, NB, H + 2, W + 2), f32)
    xp[:, :, 1:-1, 1:-1] = x
    cols = np.empty((NB, 3, 3, B, L), f32)
    for dy in range(3):
        for dx in range(3):
            cols[:, dy, dx] = (
                xp[:, :, dy : dy + H, dx : dx + W]
                .reshape(B, NB, L).transpose(1, 0, 2))
    cols_b = cols.reshape(45, B, L)
    enc_w2 = np.ascontiguousarray(enc_w.reshape(HD, 45).T)

    ip_tap = np.empty((HD, NL, DC, DI), f32)
    ip_z = np.empty((HD, NL, DI), f32)
    wd_T = np.empty((DI, NL, DI), f32)
    bc_T = np.empty((DI, NL, 2 * DS), f32)
    a_cols = np.empty((DI, NL, DS), f32)
    op_T = np.empty((DI, NL, HD), f32)
    for i in range(NL):
        for k in range(DC):
            ip_tap[:, i, k, :] = (conv_w[i][:, k : k + 1] * in_proj[i][:DI]).T
        ip_z[:, i, :] = in_proj[i][DI:].T
        wd_T[:, i, :] = (dt_w[i] @ x_proj[i][:DTR]).T
        bc_T[:, i, :] = x_proj[i][DTR:].T
        a_cols[:, i, :] = np.exp(A_log[i])
        op_T[:, i, :] = out_proj[i].T

    dec1_tap = np.empty((HD, 9, HD), f32)
    dec2_tap = np.empty((HD, 9, NF), f32)
    for ti in range(9):
        dy, dx = ti // 3, ti % 3
        dec1_tap[:, ti, :] = dec1_w[:, :, dy, dx].T
        dec2_tap[:, ti, :] = dec2_w[:, :, dy, dx].T

    common = {
        "enc_w2": enc_w2.astype(np.float16),
        "enc_g": np.asarray(inputs["enc_g"], f32).reshape(HD, 1),
        "enc_be": np.asarray(inputs["enc_be"], f32).reshape(HD, 1),
        "ip_tap": ip_tap.reshape(HD, NL * DC * DI).astype(np.float16),
        "ip_z": ip_z.reshape(HD, NL * DI).astype(np.float16),
        "conv_b": np.ascontiguousarray(
            np.asarray(inputs["conv_b"], f32).T),           # (DI, NL)
        "wd_T": wd_T.reshape(DI, NL * DI).astype(bf16),
        "bc_T": bc_T.reshape(DI, NL * 2 * DS).astype(bf16),
        "dt_b": np.ascontiguousarray(-np.asarray(inputs["dt_b"], f32).T),
        "a_cols": a_cols.reshape(DI, NL * DS),
        "d_col": np.ascontiguousarray(np.asarray(inputs["Dp"], f32).T),
        "op_T": op_T.reshape(DI, NL * HD).astype(bf16),
        "dec1_tap": dec1_tap.reshape(HD, 9 * HD).astype(np.float16),
        "dec1_g": np.asarray(inputs["dec1_g"], f32).reshape(HD, 1),
        "dec1_be": np.asarray(inputs["dec1_be"], f32).reshape(HD, 1),
        "dec2_tap": dec2_tap.reshape(HD, 9 * NF).astype(np.float16),
        "dec2_b": np.asarray(inputs["dec2_b"], f32).reshape(NF, 1),
    }
    in_maps = []
    for c in range(8):
        b0 = c % B
        order = [b0] + [bb for bb in range(B) if bb != b0]
        m = dict(common)
        m["enc_im2col"] = np.ascontiguousarray(
            cols_b[:, order, :].reshape(45, B * L)).astype(np.float16)
        in_maps.append(m)
    return in_maps


def kernel(**inputs):
    nc = _get_kernel()
    in_maps = _host_inputs(inputs)
    res = run_bass_kernel_spmd(nc, in_maps, core_ids=list(range(8)))
    out = np.empty((B, NF, H, W), np.float32)
    for b_ in range(B):
        out[b_] = res.results[b_]["out"].reshape(NF, H, W)
    return out


if __name__ == "__main__":
    sys.path.insert(0, "/root/problem")
    import reference as ref

    inp = {k: np.asarray(v) for k, v in ref.setup_inputs().items()}
    got = kernel(**inp)
    print("kernel ran, output shape:", got.shape)


# revision 17
# speedup vs baseline: 1.6268x; 1.6238x over previous
"""Trainium2 Bass kernel for MinimalEventMamba.

kernel(**inputs) takes FULL inputs (as from setup_inputs()) and returns the
FULL (4, 10, 64, 64) float32 output. Internally: batch-parallel across 8
NeuronCores (4 batches x2 replicated), one SPMD Bass kernel launch, host
assembles the output.

Per-core layout: channel-on-partition, L=4096 on free dim.
- encoder conv as im2col matmul (all 4 batches locally -> exact BN stats)
- mamba trunk: depthwise conv folded into in_proj taps (shifted matmuls,
  PSUM accumulate), dt_w folded into x_proj, softplus/silu/exp fused into
  PSUM evictions on ScalarE (per-partition scale/bias), selective scan via
  tensor_tensor_scan on DVE per state dim, B/C rows broadcast across
  partitions via DRAM-bounce DMA, y accumulation split DVE/GpSimd.
- decoder convs in zero-padded spatial layout (row stride 66) as 9-tap
  PSUM-accumulated matmuls; cross-batch BN stats via one tiny AllReduce.
"""
import sys
import types

sys.path.insert(0, "/opt/trn_rl_repo")
sys.path.insert(0, "/opt/trn_rl_repo/concourse")
try:
    from antenv import axon_hooks  # noqa: F401
except ImportError:
    try:
        from trn_agent_boot.trn_boot import _ntff_profile_via_ctypes
        _m = types.ModuleType("antenv.axon_hooks")
        _h = _ntff_profile_via_ctypes("/opt/axon/libaxon_pjrt.so")
        _m.get_axon_ntff_profile_hook = lambda: _h
        _m.set_axon_ntff_profile_hook = lambda h: None
        sys.modules["antenv.axon_hooks"] = _m
    except Exception:
        pass

from contextlib import ExitStack

import numpy as np
import ml_dtypes

import concourse.bass as bass
import concourse.tile as tile
from concourse import mybir
from concourse.bass_utils import run_bass_kernel_spmd
import bass_rust

F32 = mybir.dt.float32
F32R = mybir.dt.float32r
BF16 = mybir.dt.bfloat16
FP16 = mybir.dt.float16

NB, HD, NL, NF = 5, 64, 4, 10
DI, DS, DC, DTR = 128, 16, 4, 4
B, H, W = 4, 64, 64
L = H * W                     # 4096
PW = W + 2                    # padded row stride 66
PADL = PW * (H + 2) + 4       # padded spatial + guard cols (4360)
PBASE = 1 + PW + 1            # first interior col in padded layout
NCHUNK = 8                    # L / 512
CS = 512
LH = L // 2                   # s-loop half length
DSL = DS // 2                 # states per core (s-split across core pairs)


def split_excess_waits(nc, max_waits=1):
    """This container's walrus accepts only 1 sync wait per instruction;
    move overflow waits onto NOPs inserted before the offending op."""
    f = nc.m.functions[0]
    for bb in f.blocks:
        insts = bb.instructions
        i = 0
        while i < len(insts):
            inst = insts[i]
            si = inst.sync_info
            if si is not None and len(si.on_wait) > max_waits:
                waits = list(si.on_wait)
                si.on_wait = waits[-max_waits:]
                inst.sync_info = si
                overflow = waits[:-max_waits]
                eng = nc.engines[inst.engine]
                pos = i
                for j in range(0, len(overflow), max_waits):
                    nop = eng.nop(hint="splitw", nofuse=True)
                    nop_inst = nop.ins if hasattr(nop, "ins") else nop
                    for bb2 in f.blocks:
                        if any(x is nop_inst for x in bb2.instructions):
                            bb2.instructions[:] = [
                                x for x in bb2.instructions if x is not nop_inst
                            ]
                            break
                    nop_inst.sync_info = bass_rust.SyncInfo(
                        on_wait=overflow[j : j + max_waits], on_update=[]
                    )
                    insts.insert(pos, nop_inst)
                    pos += 1
                i = pos + 1
            else:
                i += 1


def _r(ap):
    return ap.bitcast(F32R)


def build_kernel():
    nc = bass.Bass()
    dp = nc.declare_dram_parameter

    enc_in = dp("enc_im2col", [45, B * L], FP16, isOutput=False)
    enc_w2 = dp("enc_w2", [45, HD], FP16, isOutput=False)
    enc_g = dp("enc_g", [HD, 1], F32, isOutput=False)
    enc_be = dp("enc_be", [HD, 1], F32, isOutput=False)
    ip_tap = dp("ip_tap", [HD, NL * DC * DI], FP16, isOutput=False)
    ip_z = dp("ip_z", [HD, NL * DI], FP16, isOutput=False)
    conv_b = dp("conv_b", [DI, NL], F32, isOutput=False)
    wd_T = dp("wd_T", [DI, NL * DI], BF16, isOutput=False)
    bc_T = dp("bc_T", [DI, NL * 2 * DSL], BF16, isOutput=False)
    dt_b = dp("dt_b", [DI, NL], F32, isOutput=False)
    a_cols = dp("a_cols", [DI, NL * DSL], F32, isOutput=False)
    d_col = dp("d_col", [DI, NL], F32, isOutput=False)
    op_T = dp("op_T", [DI, NL * HD], BF16, isOutput=False)
    dec1_tap = dp("dec1_tap", [HD, 9 * HD], FP16, isOutput=False)
    dec1_g = dp("dec1_g", [HD, 1], F32, isOutput=False)
    dec1_be = dp("dec1_be", [HD, 1], F32, isOutput=False)
    dec2_tap = dp("dec2_tap", [HD, 9 * NF], FP16, isOutput=False)
    dec2_b = dp("dec2_b", [NF, 1], F32, isOutput=False)

    out_ext = dp("out", [NF, L], F32, isOutput=True)

    bc_dram = nc.dram_tensor("bc_dram", [NL, 2 * DSL, L], BF16)
    y_in = nc.dram_tensor("y_in", [DI, L], BF16)
    y_out = nc.dram_tensor("y_out", [DI, L], BF16)
    cc_in = nc.dram_tensor("cc_in", [HD, 2], F32)
    cc_out = nc.dram_tensor("cc_out", [HD, 2], F32, addr_space="Shared")

    ctx = ExitStack()
    with ctx:
        tc = ctx.enter_context(tile.TileContext(nc))
        const = ctx.enter_context(tc.tile_pool(name="const", bufs=1))
        persist = ctx.enter_context(tc.tile_pool(name="persist", bufs=1))
        work = ctx.enter_context(tc.tile_pool(name="work", bufs=1))
        stream = ctx.enter_context(tc.tile_pool(name="stream", bufs=2))
        sloop = ctx.enter_context(tc.tile_pool(name="sloop", bufs=2))
        small = ctx.enter_context(tc.tile_pool(name="small", bufs=1))
        psum = ctx.enter_context(tc.tile_pool(name="psum", bufs=6, space="PSUM"))
        psum1 = psum

        MM = nc.tensor.matmul
        AF = mybir.ActivationFunctionType
        OP = mybir.AluOpType
        X = mybir.AxisListType

        # ---------------- encoder ----------------
        enc_w_t = const.tile([45, HD], FP16)
        nc.sync.dma_start(enc_w_t[:], enc_w2[:])
        enc_g_t = const.tile([HD, 1], F32)
        nc.sync.dma_start(enc_g_t[:], enc_g[:])
        enc_be_t = const.tile([HD, 1], F32)
        nc.sync.dma_start(enc_be_t[:], enc_be[:])

        enc_keep = persist.tile([HD, L], F32)      # own-batch conv out
        s1p = small.tile([HD, 32], F32, tag="s1p")
        s2p = small.tile([HD, 32], F32, tag="s2p")
        for n in range(32):
            cin = stream.tile([45, CS], FP16, tag="enc_cin")
            nc.sync.dma_start(cin[:], enc_in[:, bass.ts(n, CS)])
            pt = psum.tile([HD, CS], F32, tag="mm512")
            MM(pt[:], enc_w_t[:], cin[:], start=True, stop=True)
            if n < NCHUNK:
                dst = enc_keep[:, bass.ts(n, CS)]
            else:
                scratch = stream.tile([HD, CS], F32, tag="enc_scr")
                dst = scratch[:]
            nc.scalar.activation(dst, pt[:], AF.Copy,
                                 accum_out=s1p[:, n : n + 1])
            sq = stream.tile([HD, CS], F32, tag="enc_scr")
            nc.scalar.activation(sq[:], pt[:], AF.Square,
                                 accum_out=s2p[:, n : n + 1])
        s1 = small.tile([HD, 1], F32, tag="s1")
        s2 = small.tile([HD, 1], F32, tag="s2")
        nc.vector.tensor_reduce(s1[:], s1p[:], axis=X.X, op=OP.add)
        nc.vector.tensor_reduce(s2[:], s2p[:], axis=X.X, op=OP.add)

        def bn_scale_bias(s1ap, s2ap, n_elems, g_ap, be_ap, tag):
            inv_n = 1.0 / n_elems
            mean = small.tile([HD, 1], F32, tag=tag + "m")
            nc.vector.tensor_scalar_mul(mean[:], s1ap, inv_n)
            m2 = small.tile([HD, 1], F32, tag=tag + "m2")
            nc.vector.tensor_tensor(m2[:], mean[:], mean[:], OP.mult)
            var = small.tile([HD, 1], F32, tag=tag + "v")
            nc.vector.scalar_tensor_tensor(var[:], s2ap, inv_n, m2[:],
                                           OP.mult, OP.subtract)
            veps = small.tile([HD, 1], F32, tag=tag + "ve")
            nc.vector.tensor_scalar_add(veps[:], var[:], 1e-5)
            rv = small.tile([HD, 1], F32, tag=tag + "rv")
            nc.vector.reciprocal(rv[:], veps[:])
            rstd = small.tile([HD, 1], F32, tag=tag + "rs")
            nc.scalar.activation(rstd[:], rv[:], AF.Sqrt)
            scale = small.tile([HD, 1], F32, tag=tag + "sc")
            nc.vector.tensor_tensor(scale[:], g_ap, rstd[:], OP.mult)
            nscale = small.tile([HD, 1], F32, tag=tag + "ns")
            nc.vector.tensor_scalar_mul(nscale[:], scale[:], -1.0)
            bias = small.tile([HD, 1], F32, tag=tag + "bi")
            nc.vector.scalar_tensor_tensor(bias[:], mean[:], nscale[:], be_ap,
                                           OP.mult, OP.add)
            return scale, bias

        sc0, bi0 = bn_scale_bias(s1[:], s2[:], B * L, enc_g_t[:], enc_be_t[:],
                                 "bn0")

        t_t = persist.tile([HD, 4 + L], F32)
        nc.vector.memset(t_t[:, 0:4], 0.0)
        nc.scalar.activation(t_t[:, 4:], enc_keep[:], AF.Relu,
                             bias=bi0[:], scale=sc0[:])
        t16 = persist.tile([HD, 4 + L], FP16)
        nc.vector.memset(t16[:, 0:4], 0.0)
        nc.scalar.activation(t16[:, 4:], t_t[:, 4:], AF.Copy)

        # ---------------- trunk weights ----------------
        iptap_t = const.tile([HD, NL * DC * DI], FP16)
        nc.sync.dma_start(iptap_t[:], ip_tap[:])
        ipz_t = const.tile([HD, NL * DI], FP16)
        nc.sync.dma_start(ipz_t[:], ip_z[:])
        convb_t = const.tile([DI, NL], F32)
        nc.sync.dma_start(convb_t[:], conv_b[:])
        wd_t = const.tile([DI, NL * DI], BF16)
        nc.sync.dma_start(wd_t[:], wd_T[:])
        bct_t = const.tile([DI, NL * 2 * DSL], BF16)
        nc.sync.dma_start(bct_t[:], bc_T[:])
        dtb_t = const.tile([DI, NL], F32)
        nc.sync.dma_start(dtb_t[:], dt_b[:])
        acols_t = const.tile([DI, NL * DSL], F32)
        nc.sync.dma_start(acols_t[:], a_cols[:])
        dcol_t = const.tile([DI, NL], F32)
        nc.sync.dma_start(dcol_t[:], d_col[:])
        opt_t = const.tile([DI, NL * HD], BF16)
        nc.sync.dma_start(opt_t[:], op_T[:])
        bc_sign = const.tile([2 * DSL, 1], F32)
        nc.vector.memset(bc_sign[:], 1.0)
        nc.vector.memset(bc_sign[0:DSL], -1.0)

        # ---------------- trunk ----------------
        for li in range(NL):
            xi_c = work.tile([DI, L], BF16, tag="xi_c")
            sz = work.tile([DI, L], BF16, tag="sz")
            dlt = work.tile([DI, L], BF16, tag="dlt")
            bc_sb = work.tile([2 * DSL, L], BF16, tag="bc_sb")
            for n in range(NCHUNK):
                p_xi = psum.tile([DI, CS], F32, tag="mm512")
                for k in range(DC):
                    MM(p_xi[:],
                       iptap_t[:, (li * DC + k) * DI : (li * DC + k + 1) * DI],
                       t16[:, 1 + k + n * CS : 1 + k + n * CS + CS],
                       start=(k == 0), stop=(k == DC - 1))
                nc.scalar.activation(xi_c[:, bass.ts(n, CS)], p_xi[:], AF.Silu,
                                     bias=convb_t[:, li : li + 1], scale=1.0)
                p_z = psum.tile([DI, CS], F32, tag="mm512")
                MM(p_z[:], ipz_t[:, li * DI : (li + 1) * DI],
                   t16[:, 4 + n * CS : 4 + (n + 1) * CS],
                   start=True, stop=True)
                nc.scalar.activation(sz[:, bass.ts(n, CS)], p_z[:], AF.Silu)
            for n in range(NCHUNK):
                p_d = psum.tile([DI, CS], F32, tag="mm512")
                MM(p_d[:], wd_t[:, li * DI : (li + 1) * DI],
                   xi_c[:, bass.ts(n, CS)], start=True, stop=True)
                # delta = softplus(p_d + dt_b); store dlt = -delta = ln(sigmoid(-(p_d+dt_b)))
                sgm = stream.tile([DI, CS], F32, tag="sgm")
                nc.scalar.activation(sgm[:], p_d[:], AF.Sigmoid,
                                     bias=dtb_t[:, li : li + 1], scale=-1.0)
                nc.scalar.activation(dlt[:, bass.ts(n, CS)], sgm[:], AF.Ln)
                p_bc = psum1.tile([2 * DSL, CS], F32, tag="mm512")
                MM(p_bc[:], bct_t[:, li * 2 * DSL : (li + 1) * 2 * DSL],
                   xi_c[:, bass.ts(n, CS)], start=True, stop=True)
                nc.scalar.activation(bc_sb[:, bass.ts(n, CS)], p_bc[:], AF.Copy,
                                     scale=bc_sign[:])
            nc.sync.dma_start(bc_dram[li], bc_sb[:])

            du = work.tile([DI, L], BF16, tag="du")
            nc.vector.tensor_tensor(du[:], dlt[:], xi_c[:], OP.mult)
            # per-half ping-pong accumulators (out-of-place keeps DVE 2x mode)
            acc = [None, None]
            for hf in range(2):
                a0 = sloop.tile([DI, LH], BF16, tag=f"acc{hf}")
                nc.vector.tensor_scalar_mul(
                    a0[:], xi_c[:, hf * LH : (hf + 1) * LH],
                    dcol_t[:, li : li + 1])
                acc[hf] = a0

            for s in range(DSL):
                hs_prev = None
                for hf in range(2):
                    sl = slice(hf * LH, (hf + 1) * LH)
                    dA = sloop.tile([DI, LH], FP16, tag="dA")
                    nc.scalar.activation(
                        dA[:], dlt[:, sl], AF.Exp,
                        scale=acols_t[:, li * DSL + s : li * DSL + s + 1])
                    brep = sloop.tile([DI, LH], BF16, tag="brep")
                    nc.sync.dma_start(
                        brep[:],
                        bc_dram[li][s : s + 1, sl].broadcast_to((DI, LH)))
                    crep = sloop.tile([DI, LH], BF16, tag="crep")
                    nc.sync.dma_start(
                        crep[:],
                        bc_dram[li][DSL + s : DSL + s + 1, sl].broadcast_to(
                            (DI, LH)))
                    xs = sloop.tile([DI, LH], BF16, tag="xs")
                    nc.vector.tensor_tensor(xs[:], du[:, sl], brep[:], OP.mult)
                    hs = sloop.tile([DI, LH], BF16, tag="hs")
                    init = 0.0 if hf == 0 else hs_prev[:, LH - 1 : LH]
                    nc.vector.tensor_tensor_scan(hs[:], dA[:], xs[:], init,
                                                 OP.mult, OP.add)
                    hs_prev = hs
                    hc = sloop.tile([DI, LH], BF16, tag="hc")
                    nc.vector.tensor_tensor(hc[:], hs[:], crep[:], OP.mult)
                    anew = sloop.tile([DI, LH], BF16, tag=f"acc{hf}")
                    nc.vector.tensor_tensor(anew[:], acc[hf][:], hc[:], OP.add)
                    acc[hf] = anew

            for hf in range(2):
                nc.sync.dma_start(y_in[:, hf * LH : (hf + 1) * LH], acc[hf][:])
            nc.gpsimd.collective_compute(
                "AllReduce", OP.add,
                replica_groups=[[0, 4], [1, 5], [2, 6], [3, 7]],
                ins=[y_in[:]], outs=[y_out[:]])
            ysum = work.tile([DI, L], BF16, tag="dlt")  # dlt dead by now; reuse slot
            nc.sync.dma_start(ysum[:], y_out[:])
            yg = work.tile([DI, L], BF16, tag="du")  # du dead by now; reuse slot
            nc.vector.tensor_tensor(yg[:], ysum[:], sz[:], OP.mult)
            for n in range(NCHUNK):
                p_o = psum1.tile([HD, CS], F32, tag="mm512")
                MM(p_o[:], opt_t[:, li * HD : (li + 1) * HD],
                   yg[:, bass.ts(n, CS)], start=True, stop=True)
                nc.vector.tensor_tensor(
                    t_t[:, 4 + n * CS : 4 + (n + 1) * CS],
                    t_t[:, 4 + n * CS : 4 + (n + 1) * CS], p_o[:], OP.add)
                if li < NL - 1:
                    nc.scalar.activation(
                        t16[:, 4 + n * CS : 4 + (n + 1) * CS],
                        t_t[:, 4 + n * CS : 4 + (n + 1) * CS], AF.Copy)

        # ---------------- decoder ----------------
        d1_taps = const.tile([HD, 9 * HD], FP16)
        nc.sync.dma_start(d1_taps[:], dec1_tap[:])
        d2_taps = const.tile([HD, 9 * NF], FP16)
        nc.sync.dma_start(d2_taps[:], dec2_tap[:])
        d1g_t = const.tile([HD, 1], F32)
        nc.sync.dma_start(d1g_t[:], dec1_g[:])
        d1be_t = const.tile([HD, 1], F32)
        nc.sync.dma_start(d1be_t[:], dec1_be[:])
        d2b_t = const.tile([NF, 1], F32)
        nc.sync.dma_start(d2b_t[:], dec2_b[:])

        padA = persist.tile([HD, PADL], FP16)
        nc.vector.memset(padA[:], 0.0)
        padB = persist.tile([HD, PADL], FP16)
        nc.vector.memset(padB[:], 0.0)
        out_pad = persist.tile([NF, PADL], F32)

        def interior(tile_ap):
            return tile_ap[:, PBASE : PBASE + PW * H].rearrange(
                "p (h w) -> p h w", w=PW)[:, :, 0:W]

        nc.scalar.activation(interior(padA),
                             t_t[:, 4:].rearrange("p (h w) -> p h w", w=W),
                             AF.Copy)

        def conv9(dst_tile, src_tile, taps_tile, m_out, tapw, evict):
            total = PW * H
            nch = (total + CS - 1) // CS
            for n in range(nch):
                c0 = PBASE + n * CS
                cw = min(CS, PBASE + total - c0)
                pt = psum1.tile([m_out, CS], F32, tag="mm512")
                for ti in range(9):
                    dy, dx = ti // 3, ti % 3
                    off = c0 + (dy - 1) * PW + (dx - 1)
                    MM(pt[:, 0:cw],
                       taps_tile[:, ti * tapw : ti * tapw + m_out],
                       src_tile[:, off : off + cw],
                       start=(ti == 0), stop=(ti == 8))
                evict(dst_tile[0:m_out, c0 : c0 + cw], pt[:, 0:cw])

        conv9(padB, padA, d1_taps, HD, HD,
              lambda d, p: nc.scalar.activation(d, p, AF.Copy))

        d1_int = interior(padB)
        ds1 = small.tile([HD, 1], F32, tag="ds1")
        nc.vector.tensor_reduce(ds1[:], d1_int, axis=X.XY, op=OP.add)
        ds2 = small.tile([HD, 1], F32, tag="ds2")
        nc.scalar.activation(interior(padA), d1_int, AF.Square,
                             accum_out=ds2[:])
        packed = small.tile([HD, 2], F32, tag="pk")
        nc.vector.tensor_copy(packed[:, 0:1], ds1[:])
        nc.vector.tensor_copy(packed[:, 1:2], ds2[:])
        nc.sync.dma_start(cc_in[:], packed[:])
        nc.gpsimd.collective_compute(
            "AllReduce", OP.add, replica_groups=[list(range(8))],
            ins=[cc_in[:]], outs=[cc_out[:]])
        red = small.tile([HD, 2], F32, tag="red")
        nc.sync.dma_start(red[:], cc_out[:])
        sc1, bi1 = bn_scale_bias(red[:, 0:1], red[:, 1:2], 2 * B * L,
                                 d1g_t[:], d1be_t[:], "bn1")

        # h2 into padA (pads still zero outside interior; square scratch
        # gets overwritten)
        nc.scalar.activation(interior(padA), d1_int, AF.Relu,
                             bias=bi1[:], scale=sc1[:])
        # conv2 into padB (its pads hold stale conv1 data only at pad
        # columns; interior fully rewritten; final DMA reads interior only)
        conv9(out_pad, padA, d2_taps, NF, NF,
              lambda d, p: nc.scalar.activation(
                  d, p, AF.Identity, bias=d2b_t[:], scale=1.0))
        out_int = out_pad[:NF, PBASE : PBASE + PW * H].rearrange(
            "p (h w) -> p h w", w=PW)[:, :, 0:W]
        nc.sync.dma_start(out_ext[:].rearrange("p (h w) -> p h w", w=W),
                          out_int)

    split_excess_waits(nc)
    return nc


_CACHED = {}


def _get_kernel():
    if "nc" not in _CACHED:
        _CACHED["nc"] = build_kernel()
    return _CACHED["nc"]


def _host_inputs(inputs):
    f32 = np.float32
    bf16 = ml_dtypes.bfloat16
    x = np.asarray(inputs["x"], f32)
    enc_w = np.asarray(inputs["enc_w"], f32)
    in_proj = np.asarray(inputs["in_proj"], f32)
    conv_w = np.asarray(inputs["conv_w"], f32)
    x_proj = np.asarray(inputs["x_proj"], f32)
    dt_w = np.asarray(inputs["dt_w"], f32)
    A_log = np.asarray(inputs["A_log"], f32)
    out_proj = np.asarray(inputs["out_proj"], f32)
    dec1_w = np.asarray(inputs["dec1_w"], f32)
    dec2_w = np.asarray(inputs["dec2_w"], f32)

    xp = np.zeros((B, NB, H + 2, W + 2), f32)
    xp[:, :, 1:-1, 1:-1] = x
    cols = np.empty((NB, 3, 3, B, L), f32)
    for dy in range(3):
        for dx in range(3):
            cols[:, dy, dx] = (
                xp[:, :, dy : dy + H, dx : dx + W]
                .reshape(B, NB, L).transpose(1, 0, 2))
    cols_b = cols.reshape(45, B, L)
    enc_w2 = np.ascontiguousarray(enc_w.reshape(HD, 45).T)

    ip_tap = np.empty((HD, NL, DC, DI), f32)
    ip_z = np.empty((HD, NL, DI), f32)
    wd_T = np.empty((DI, NL, DI), f32)
    bc_full = np.empty((DI, NL, 2 * DS), f32)
    a_full = np.empty((DI, NL, DS), f32)
    op_T = np.empty((DI, NL, HD), f32)
    for i in range(NL):
        for k in range(DC):
            ip_tap[:, i, k, :] = (conv_w[i][:, k : k + 1] * in_proj[i][:DI]).T
        ip_z[:, i, :] = in_proj[i][DI:].T
        wd_T[:, i, :] = (dt_w[i] @ x_proj[i][:DTR]).T
        bc_full[:, i, :] = x_proj[i][DTR:].T
        a_full[:, i, :] = np.exp(A_log[i])
        op_T[:, i, :] = out_proj[i].T

    dec1_tap = np.empty((HD, 9, HD), f32)
    dec2_tap = np.empty((HD, 9, NF), f32)
    for ti in range(9):
        dy, dx = ti // 3, ti % 3
        dec1_tap[:, ti, :] = dec1_w[:, :, dy, dx].T
        dec2_tap[:, ti, :] = dec2_w[:, :, dy, dx].T

    common = {
        "enc_w2": enc_w2.astype(np.float16),
        "enc_g": np.asarray(inputs["enc_g"], f32).reshape(HD, 1),
        "enc_be": np.asarray(inputs["enc_be"], f32).reshape(HD, 1),
        "ip_tap": ip_tap.reshape(HD, NL * DC * DI).astype(np.float16),
        "ip_z": ip_z.reshape(HD, NL * DI).astype(np.float16),
        "conv_b": np.ascontiguousarray(
            np.asarray(inputs["conv_b"], f32).T),           # (DI, NL)
        "wd_T": wd_T.reshape(DI, NL * DI).astype(bf16),

        "dt_b": np.ascontiguousarray(-np.asarray(inputs["dt_b"], f32).T),

        "d_col": np.ascontiguousarray(np.asarray(inputs["Dp"], f32).T) / 2.0,
        "op_T": op_T.reshape(DI, NL * HD).astype(bf16),
        "dec1_tap": dec1_tap.reshape(HD, 9 * HD).astype(np.float16),
        "dec1_g": np.asarray(inputs["dec1_g"], f32).reshape(HD, 1),
        "dec1_be": np.asarray(inputs["dec1_be"], f32).reshape(HD, 1),
        "dec2_tap": dec2_tap.reshape(HD, 9 * NF).astype(np.float16),
        "dec2_b": np.asarray(inputs["dec2_b"], f32).reshape(NF, 1),
    }
    in_maps = []
    for c in range(8):
        b0 = c % B
        sr = (c // B) * DSL
        order = [b0] + [bb for bb in range(B) if bb != b0]
        m = dict(common)
        m["enc_im2col"] = np.ascontiguousarray(
            cols_b[:, order, :].reshape(45, B * L)).astype(np.float16)
        bcs = np.concatenate(
            [bc_full[:, :, sr : sr + DSL],
             bc_full[:, :, DS + sr : DS + sr + DSL]], axis=2)
        m["bc_T"] = np.ascontiguousarray(
            bcs.reshape(DI, NL * 2 * DSL)).astype(bf16)
        m["a_cols"] = np.ascontiguousarray(
            a_full[:, :, sr : sr + DSL].reshape(DI, NL * DSL))
        in_maps.append(m)
    return in_maps


def kernel(**inputs):
    nc = _get_kernel()
    in_maps = _host_inputs(inputs)
    res = run_bass_kernel_spmd(nc, in_maps, core_ids=list(range(8)))
    out = np.empty((B, NF, H, W), np.float32)
    for b_ in range(B):
        out[b_] = res.results[b_]["out"].reshape(NF, H, W)
    return out


if __name__ == "__main__":
    sys.path.insert(0, "/root/problem")
    import reference as ref

    inp = {k: np.asarray(v) for k, v in ref.setup_inputs().items()}
    got = kernel(**inp)
    print("kernel ran, output shape:", got.shape)
